# revision 1
# baseline (speedup 1.0000x reference)
"""Trainium2 Bass kernel for geodesic convolution (gnn_message_passing).

Computation (per vertex v):
  x[v,i,j,c]  = sum_t bary_w[v,i,j,t] * signal[bary_idx[v,i,j,t], c]
  conv[v,k,d] = sum_{i,j,c} x[v,i,j,c] * K[i,(j+k)%NT,c,d]
  out[v,:]    = relu(conv[v, argmax_k ||conv[v,k,:]||, :])

Strategy: shard V across 8 cores (data-parallel). Per core, per tile of 128
vertices: dma_gather of the 3*5*8 = 120 signal rows per vertex (v-major
layout; int16 indices biased by -32768 against a mid-tensor base so the
signed offsets cover all 50000 rows), DVE weighted sum over the 3 barycentric
taps, PE transpose of x to channel-major, one accumulated matmul chain
against the pre-rotated kernel matrix W[(i,j,c),(k,d)], then
norms/argmax/select/relu epilogue on DVE.
"""

import numpy as np

# Problem constants (hardcoded; kernel.py must be self-contained).
V, NR, NT, CIN, COUT = 50000, 5, 8, 64, 64
NCORES = 8
VPC = V // NCORES            # 6250 vertices per core
TPT = 128                    # vertices per tile (partition dim)
NTILES = -(-VPC // TPT)      # 49
VPAD = NTILES * TPT          # 6272
IJ = NR * NT                 # 40
E = IJ * 3                   # 120 gathered rows per vertex
EP = E + 1                   # +1 pad slot per partition (trailing-trim guard)
NIDX = EP * TPT              # 15488 gather indices per tile
NS = NIDX // 16              # idx free dim in wrapped-16 layout
KC = IJ * CIN                # 2560 contraction dim
NCHUNK = KC // 128           # 20
ND = NT * COUT               # 512 output cols (k,d)

_CACHE = {}


def build_program(ntiles=NTILES, v_src=V, repeat=1):
    """Build the Bacc program for one SPMD core. Returns compiled nc.

    repeat > 1 duplicates the whole tile loop (same inputs/outputs) for
    wall-clock slope timing; the extra passes just overwrite the outputs.
    """
    import concourse.bass as bass
    import concourse.mybir as mybir
    import concourse.tile as tile
    from concourse import bacc
    from concourse.masks import make_identity

    f32 = mybir.dt.float32
    i16 = mybir.dt.int16

    base = 32768 if v_src > 32768 else 0

    nc = bacc.Bacc(
        "TRN2",
        target_bir_lowering=False,
        debug=False,
        enable_asserts=False,
        num_devices=NCORES,
    )
    vpad = ntiles * TPT
    sig_d = nc.dram_tensor("signal", [v_src, CIN], f32, kind="ExternalInput")
    wv_d = nc.dram_tensor("wv", [vpad, E], f32, kind="ExternalInput")
    idx_d = nc.dram_tensor("idx16", [ntiles * 128, NS], i16, kind="ExternalInput")
    wm_d = nc.dram_tensor("wm", [KC, ND], f32, kind="ExternalInput")
    out_d = nc.dram_tensor("out", [vpad, COUT], f32, kind="ExternalOutput")

    sig_base = sig_d.ap()[base:, :] if base else sig_d.ap()

    TB = 7  # tiles per input-DMA batch (49 = 7*7)
    with tile.TileContext(nc) as tc:
        with (
            tc.tile_pool(name="const", bufs=1) as cpool,
            tc.tile_pool(name="io", bufs=2) as iopool,
            tc.tile_pool(name="g", bufs=2) as gpool,
            tc.tile_pool(name="x", bufs=2) as xpool,
            tc.tile_pool(name="xT", bufs=2) as xtpool,
            tc.tile_pool(name="epi", bufs=1) as epool,
            tc.tile_pool(name="psA", bufs=2, space="PSUM") as psA,
            tc.tile_pool(name="psB", bufs=1, space="PSUM") as psB,
        ):
            # Resident: rotated kernel matrix [128, NCHUNK, 512] (chunk k of
            # contraction rows at [:, k, :]) and transpose identity.
            wm_t = cpool.tile([128, NCHUNK, ND], f32)
            nc.sync.dma_start(
                out=wm_t[:],
                in_=wm_d.ap().rearrange("(k p) n -> p k n", p=128),
            )
            ident = cpool.tile([128, 128], f32)
            make_identity(nc, ident[:])

            for it_rep in range(ntiles * repeat):
                it = it_rep % ntiles
                rows = slice(it * TPT, (it + 1) * TPT)
                bt = it % TB  # position within the input batch
                if bt == 0:
                    # Batched input DMAs covering TB tiles at once.
                    nb = min(TB, ntiles - it)
                    brows = slice(it * TPT, (it + nb) * TPT)
                    w_t = iopool.tile([128, TB, E], f32, tag="w")
                    i_t = iopool.tile([128, TB, NS], i16, tag="i")
                    nc.sync.dma_start(
                        out=w_t[:, :nb, :],
                        in_=wv_d.ap()[brows, :].rearrange("(t p) e -> p t e", p=128))
                    nc.sync.dma_start(
                        out=i_t[:, :nb, :],
                        in_=idx_d.ap()[brows, :].rearrange("(t p) s -> p t s", p=128))

                # Gather: g[p, e, :] = signal[idx[v_p, e], :]  (e < E; slot E is pad)
                g_t = gpool.tile([128, EP, CIN], f32)
                nc.gpsimd.dma_gather(
                    out_ap=g_t[:], in_ap=sig_base, idxs_ap=i_t[:, bt, :],
                    num_idxs=NIDX, num_idxs_reg=NIDX, elem_size=CIN,
                    single_packet=False,
                )

                # Weighted sum over the 3 barycentric taps.
                g_e = g_t[:, :E, :]
                nc.vector.tensor_tensor(
                    out=g_e,
                    in0=g_e,
                    in1=w_t[:, bt, :].unsqueeze(-1).to_broadcast([128, E, CIN]),
                    op=mybir.AluOpType.mult,
                )
                g4 = g_e.rearrange("p (ij t) c -> p ij t c", t=3)
                x_t = xpool.tile([128, IJ, CIN], f32)
                nc.vector.tensor_tensor(
                    out=x_t[:], in0=g4[:, :, 0, :], in1=g4[:, :, 1, :],
                    op=mybir.AluOpType.add,
                )
                nc.vector.tensor_tensor(
                    out=x_t[:], in0=x_t[:], in1=g4[:, :, 2, :],
                    op=mybir.AluOpType.add,
                )

                # All 20 PE transposes into one 5-bank PSUM tile, ONE bulk
                # PSUM->SBUF copy, then an uninterrupted 20-matmul chain.
                # (This platform is instruction-issue-bound, so fewer + fatter
                # ops and no PE<->ACT interleave matter more than engine time.)
                x2 = x_t[:].rearrange("p ij c -> p (ij c)")
                xTp = psB.tile([128, NCHUNK, 128], f32, tag="xTp")
                for k in range(NCHUNK):
                    nc.tensor.transpose(
                        xTp[:, k, :], x2[:, k * 128:(k + 1) * 128], ident[:]
                    )
                xT = xtpool.tile([128, NCHUNK, 128], f32, tag="xT")
                nc.scalar.copy(out=xT[:], in_=xTp[:])
                conv_p = psA.tile([128, ND], f32, tag="conv")
                for k in range(NCHUNK):
                    nc.tensor.matmul(
                        conv_p[:],
                        lhsT=xT[:, k, :],
                        rhs=wm_t[:, k, :],
                        start=(k == 0),
                        stop=(k == NCHUNK - 1),
                    )

                # Per-tile: square (ACT) + norm reduce (DVE) + conv copy to a
                # batch slab.  The rest of the epilogue runs once per TB tiles.
                if bt == 0:
                    conv_sb = epool.tile([128, TB, ND], f32, tag="conv_sb")
                    norm_t = epool.tile([128, TB, NT], f32, tag="norm")
                sq_t = xpool.tile([128, ND], f32, tag="sq")
                nc.scalar.activation(
                    out=sq_t[:], in_=conv_p[:],
                    func=mybir.ActivationFunctionType.Square,
                )
                nc.vector.tensor_reduce(
                    out=norm_t[:, bt, :],
                    in_=sq_t[:].rearrange("p (k d) -> p k d", d=COUT),
                    axis=mybir.AxisListType.X,
                    op=mybir.AluOpType.add,
                )
                nc.scalar.copy(out=conv_sb[:, bt, :], in_=conv_p[:])

                if bt == TB - 1 or it == ntiles - 1:
                    nb = bt + 1
                    b0 = it - bt  # first tile of this batch
                    mx_t = epool.tile([128, TB, 1], f32, tag="mx")
                    nc.vector.tensor_reduce(
                        out=mx_t[:, :nb, :], in_=norm_t[:, :nb, :],
                        axis=mybir.AxisListType.X, op=mybir.AluOpType.max,
                    )
                    mask_t = epool.tile([128, TB, NT], f32, tag="mask")
                    nc.vector.tensor_tensor(
                        out=mask_t[:, :nb, :], in0=norm_t[:, :nb, :],
                        in1=mx_t[:, :nb, :].to_broadcast([128, nb, NT]),
                        op=mybir.AluOpType.is_equal,
                    )
                    msel_v = conv_sb[:, :nb, :].rearrange(
                        "p b (k d) -> p b k d", d=COUT)
                    nc.vector.tensor_tensor(
                        out=msel_v,
                        in0=msel_v,
                        in1=mask_t[:, :nb, :].unsqueeze(-1)
                        .to_broadcast([128, nb, NT, COUT]),
                        op=mybir.AluOpType.mult,
                    )
                    o_t = epool.tile([128, TB, COUT], f32, tag="o")
                    nc.vector.tensor_reduce(
                        out=o_t[:, :nb, :],
                        in_=msel_v.rearrange("p b k d -> p b d k"),
                        axis=mybir.AxisListType.X,
                        op=mybir.AluOpType.add,
                    )
                    nc.vector.tensor_scalar_max(o_t[:, :nb, :], o_t[:, :nb, :], 0.0)
                    orows = slice(b0 * TPT, (it + 1) * TPT)
                    nc.sync.dma_start(
                        out=out_d.ap()[orows, :].rearrange(
                            "(t p) c -> p t c", p=128),
                        in_=o_t[:, :nb, :])

    nc.compile()
    return nc


def make_idx16(idx_vp, ntiles, base):
    """[vpad, E] int32 row indices -> [ntiles*128, NS] wrapped int16.

    Gather list position n = e*128 + p must hold idx[tile*128 + p, e]; the
    ucode reads logical position i from wrapped[i % 16, i // 16], replicated
    across the 8 Q7 cores (16 partitions each). Slot E is a pad row of
    positive indices so the ucode's trailing-negative trim never fires.
    """
    vpad = ntiles * TPT
    out = np.empty((ntiles, 128, NS), np.int16)
    for t in range(ntiles):
        blk = idx_vp[t * TPT:(t + 1) * TPT]          # [128, E]
        lst = np.full(NIDX, 1, np.int32)             # pad slots -> row base+1
        lst[: E * 128] = (blk.T.astype(np.int32) - base).ravel()  # n = e*128+p
        w = lst.reshape(-1, 16).T                     # [16, NIDX/16]
        out[t] = np.tile(w, (8, 1))
    return out.reshape(ntiles * 128, NS)


def _host_prep(signal, bary_w, bary_idx, kernel):
    """Build per-core input maps. All host-side numpy, not timed."""
    jj = np.arange(NT)
    rot = kernel[:, (jj[:, None] + jj[None, :]) % NT, :, :]  # [i,j,k,c,d]
    wm = np.ascontiguousarray(
        rot.transpose(0, 1, 3, 2, 4).reshape(KC, ND), dtype=np.float32
    )
    sig = np.ascontiguousarray(signal, dtype=np.float32)
    base = 32768
    wv_full = bary_w.reshape(V, E).astype(np.float32)
    idx_full = bary_idx.reshape(V, E).astype(np.int32)
    in_maps = []
    for c in range(NCORES):
        sl = slice(c * VPC, (c + 1) * VPC)
        wv = np.zeros((VPAD, E), np.float32)
        wv[:VPC] = wv_full[sl]
        idx = np.zeros((VPAD, E), np.int32)
        idx[:VPC] = idx_full[sl]
        in_maps.append({
            "signal": sig,
            "wv": wv,
            "idx16": make_idx16(idx, NTILES, base),
            "wm": wm,
        })
    return in_maps


def kernel(signal, bary_w, bary_idx, kernel):
    from concourse.bass_utils import run_bass_kernel_spmd

    if "nc" not in _CACHE:
        _CACHE["nc"] = build_program()
    nc = _CACHE["nc"]
    in_maps = _host_prep(signal, bary_w, bary_idx, kernel)
    res = run_bass_kernel_spmd(nc, in_maps, core_ids=list(range(NCORES)))
    out = np.concatenate(
        [res.results[c]["out"][:VPC] for c in range(NCORES)], axis=0
    )
    return out.astype(np.float32)



# revision 7
# speedup vs baseline: 173.4228x; 173.4228x over previous
"""Trainium2 Bass kernel for geodesic convolution (gnn_message_passing).

Computation (per vertex v):
  x[v,i,j,c]  = sum_t bary_w[v,i,j,t] * signal[bary_idx[v,i,j,t], c]
  conv[v,k,d] = sum_{i,j,c} x[v,i,j,c] * K[i,(j+k)%NT,c,d]
  out[v,:]    = relu(conv[v, argmax_k ||conv[v,k,:]||, :])

Strategy: shard V across 8 cores (data-parallel). Per core, per tile of 128
vertices: the 120-row-per-vertex signal gather is SPLIT ACROSS ALL 4 SWDGE
QUEUES (each queue = its own Q7 tx/rx core pair + descriptor rings, so the
four gathers' descriptor generation and HBM drains run concurrently; measured
~4.4x over a single-queue gather, which is descriptor-rate-bound at ~9.4
ns/descriptor/queue). Each queue gathers 31 slots per vertex (30 real + 1
trailing positive pad so the ucode's trailing-negative-index trim can never
drop real rows; int16 indices are biased by -32768 against a mid-tensor
base). DVE does the weighted 3-tap sum, PE transposes x to channel-major and
runs one accumulated matmul chain against the pre-rotated kernel matrix
W[(i,j,c),(k,d)] (f32: bf16 would flip the argmax rotation selection at
near-ties and blow the error budget), then norms/argmax/select/relu epilogue on DVE.
"""

import numpy as np

# Problem constants (hardcoded; kernel.py must be self-contained).
V, NR, NT, CIN, COUT = 50000, 5, 8, 64, 64
NCORES = 8
VPC = V // NCORES            # 6250 vertices per core
TPT = 128                    # vertices per tile (partition dim)
NTILES = -(-VPC // TPT)      # 49
VPAD = NTILES * TPT          # 6272
IJ = NR * NT                 # 40
E = IJ * 3                   # 120 gathered rows per vertex
NQ = 4                       # SWDGE queues (ucode max)
SQ = E // NQ                 # 30 real slots per queue
SQP = SQ + 1                 # +1 trailing pad slot per queue
EP = NQ * SQP                # 124 gathered slots per vertex
NIDXQ = SQP * TPT            # 3968 gather indices per queue per tile
NSQ = NIDXQ // 16            # 248 idx free dim per queue (wrapped-16)
NS = NQ * NSQ                # 992 total idx free dim per tile row
KC = IJ * CIN                # 2560 contraction dim
NCHUNK = KC // 128           # 20
ND = NT * COUT               # 512 output cols (k,d)

_CACHE = {}


def build_program(ntiles=NTILES, v_src=V, repeat=1, gbufs=2, xbufs=2, split_pe=True, skip_dve=False, skip_pe=False):
    """Build the Bacc program for one SPMD core. Returns compiled nc."""
    import concourse.bass as bass
    import concourse.mybir as mybir
    import concourse.tile as tile
    from concourse import bacc
    from concourse.masks import make_identity

    f32 = mybir.dt.float32
    bf16 = mybir.dt.bfloat16
    i16 = mybir.dt.int16

    base = 32768 if v_src > 32768 else 0

    nc = bacc.Bacc(
        "TRN2",
        target_bir_lowering=False,
        debug=False,
        enable_asserts=False,
        num_devices=NCORES,
        num_swdge_queues=NQ,
    )
    vpad = ntiles * TPT
    sig_d = nc.dram_tensor("signal", [v_src, CIN], f32, kind="ExternalInput")
    wv_d = nc.dram_tensor("wv", [vpad, E], f32, kind="ExternalInput")
    idx_d = nc.dram_tensor("idx16", [ntiles * 128, NS], i16, kind="ExternalInput")
    wm_d = nc.dram_tensor("wm", [KC, ND], f32, kind="ExternalInput")
    out_d = nc.dram_tensor("out", [vpad, COUT], f32, kind="ExternalOutput")

    sig_base = sig_d.ap()[base:, :] if base else sig_d.ap()

    TB = 7  # tiles per input-DMA batch (49 = 7*7)
    with tile.TileContext(nc) as tc:
        with (
            tc.tile_pool(name="const", bufs=1) as cpool,
            tc.tile_pool(name="io", bufs=2) as iopool,
            tc.tile_pool(name="g", bufs=gbufs) as gpool,
            tc.tile_pool(name="x", bufs=xbufs) as xpool,
            tc.tile_pool(name="xT", bufs=2) as xtpool,
            tc.tile_pool(name="epi", bufs=1) as epool,
            tc.tile_pool(name="psA", bufs=2, space="PSUM") as psA,
            tc.tile_pool(name="psB", bufs=(2 if split_pe else 1), space="PSUM") as psB,
        ):
            # Resident: rotated kernel matrix [128, NCHUNK, 512] and identity.
            wm_t = cpool.tile([128, NCHUNK, ND], f32)
            nc.sync.dma_start(
                out=wm_t[:],
                in_=wm_d.ap().rearrange("(k p) n -> p k n", p=128),
            )
            ident = cpool.tile([128, 128], f32)
            make_identity(nc, ident[:])

            for it_rep in range(ntiles * repeat):
                it = it_rep % ntiles
                bt = it % TB
                if bt == 0:
                    nb = min(TB, ntiles - it)
                    brows = slice(it * TPT, (it + nb) * TPT)
                    w_t = iopool.tile([128, TB, E], f32, tag="w")
                    i_t = iopool.tile([128, TB, NS], i16, tag="i")
                    nc.sync.dma_start(
                        out=w_t[:, :nb, :],
                        in_=wv_d.ap()[brows, :].rearrange("(t p) e -> p t e", p=128))
                    nc.sync.dma_start(
                        out=i_t[:, :nb, :],
                        in_=idx_d.ap()[brows, :].rearrange("(t p) s -> p t s", p=128))

                # Gather split across the 4 SWDGE queues; queue q fills slots
                # [q*SQP, q*SQP+SQ) of g_t (slot q*SQP+SQ is its pad slot).
                g_t = gpool.tile([128, EP, CIN], f32)
                for q in range(NQ):
                    nc.gpsimd.dma_gather(
                        out_ap=g_t[:, q * SQP:(q + 1) * SQP, :],
                        in_ap=sig_base,
                        idxs_ap=i_t[:, bt, q * NSQ:(q + 1) * NSQ],
                        num_idxs=NIDXQ, num_idxs_reg=NIDXQ, elem_size=CIN,
                        single_packet=False, queue_num=q,
                    )

                # Weighted 3-tap sum. One big mult over the real slots (4d AP
                # with a stride gap over the pad slots), then per-queue adds.
                x_t = xpool.tile([128, IJ, CIN], f32)
                if skip_dve:
                    nc.scalar.copy(out=x_t[:], in_=g_t[:, :IJ, :])
                g4 = g_t[:].rearrange("p (q s) c -> p q s c", s=SQP)[:, :, :SQ, :]
                if not skip_dve:
                  nc.vector.tensor_tensor(
                    out=g4,
                    in0=g4,
                    in1=w_t[:, bt, :].rearrange("p (q s) -> p q s", s=SQ)
                    .unsqueeze(-1).to_broadcast([128, NQ, SQ, CIN]),
                    op=mybir.AluOpType.mult,
                  )
                for q in range(NQ if not skip_dve else 0):
                    g5 = g_t[:, q * SQP:q * SQP + SQ, :].rearrange(
                        "p (ij t) c -> p ij t c", t=3)
                    xq = x_t[:, q * (SQ // 3):(q + 1) * (SQ // 3), :]
                    nc.vector.tensor_tensor(
                        out=xq, in0=g5[:, :, 0, :], in1=g5[:, :, 1, :],
                        op=mybir.AluOpType.add,
                    )
                    nc.vector.tensor_tensor(
                        out=xq, in0=xq, in1=g5[:, :, 2, :],
                        op=mybir.AluOpType.add,
                    )

                # 20 PE transposes into one 5-bank PSUM tile, one bulk
                # PSUM->SBUF copy, then an uninterrupted 20-matmul chain.
                x2 = x_t[:].rearrange("p ij c -> p (ij c)")
                conv_p = psA.tile([128, ND], f32, tag="conv")
                if skip_pe:
                    nc.vector.memset(conv_p[:], 0.0)
                if skip_pe:
                    pass
                elif split_pe:
                    H = NCHUNK // 2
                    xT = xtpool.tile([128, NCHUNK, 128], f32, tag="xT")
                    for h in range(2):
                        xTp = psB.tile([128, H, 128], f32, tag="xTp")
                        for k in range(H):
                            kk = h * H + k
                            nc.tensor.transpose(
                                xTp[:, k, :], x2[:, kk * 128:(kk + 1) * 128], ident[:])
                        nc.scalar.copy(out=xT[:, h * H:(h + 1) * H, :], in_=xTp[:])
                    for k in range(NCHUNK):
                        nc.tensor.matmul(
                            conv_p[:], lhsT=xT[:, k, :], rhs=wm_t[:, k, :],
                            start=(k == 0), stop=(k == NCHUNK - 1))
                else:
                    xTp = psB.tile([128, NCHUNK, 128], f32, tag="xTp")
                    for k in range(NCHUNK):
                        nc.tensor.transpose(
                            xTp[:, k, :], x2[:, k * 128:(k + 1) * 128], ident[:]
                        )
                    xT = xtpool.tile([128, NCHUNK, 128], f32, tag="xT")
                    nc.scalar.copy(out=xT[:], in_=xTp[:])
                    for k in range(NCHUNK):
                        nc.tensor.matmul(
                            conv_p[:],
                            lhsT=xT[:, k, :],
                            rhs=wm_t[:, k, :],
                            start=(k == 0),
                            stop=(k == NCHUNK - 1),
                        )

                # Per-tile: square (ACT) + norm reduce (DVE) + conv copy to a
                # batch slab.  The rest of the epilogue runs once per TB tiles.
                if bt == 0:
                    conv_sb = epool.tile([128, TB, ND], f32, tag="conv_sb")
                    norm_t = epool.tile([128, TB, NT], f32, tag="norm")
                sq_t = xpool.tile([128, ND], f32, tag="sq")
                nc.scalar.activation(
                    out=sq_t[:], in_=conv_p[:],
                    func=mybir.ActivationFunctionType.Square,
                )
                nc.vector.tensor_reduce(
                    out=norm_t[:, bt, :],
                    in_=sq_t[:].rearrange("p (k d) -> p k d", d=COUT),
                    axis=mybir.AxisListType.X,
                    op=mybir.AluOpType.add,
                )
                nc.scalar.copy(out=conv_sb[:, bt, :], in_=conv_p[:])

                if bt == TB - 1 or it == ntiles - 1:
                    nb = bt + 1
                    b0 = it - bt  # first tile of this batch
                    mx_t = epool.tile([128, TB, 1], f32, tag="mx")
                    nc.vector.tensor_reduce(
                        out=mx_t[:, :nb, :], in_=norm_t[:, :nb, :],
                        axis=mybir.AxisListType.X, op=mybir.AluOpType.max,
                    )
                    mask_t = epool.tile([128, TB, NT], f32, tag="mask")
                    nc.vector.tensor_tensor(
                        out=mask_t[:, :nb, :], in0=norm_t[:, :nb, :],
                        in1=mx_t[:, :nb, :].to_broadcast([128, nb, NT]),
                        op=mybir.AluOpType.is_equal,
                    )
                    msel_v = conv_sb[:, :nb, :].rearrange(
                        "p b (k d) -> p b k d", d=COUT)
                    nc.vector.tensor_tensor(
                        out=msel_v,
                        in0=msel_v,
                        in1=mask_t[:, :nb, :].unsqueeze(-1)
                        .to_broadcast([128, nb, NT, COUT]),
                        op=mybir.AluOpType.mult,
                    )
                    o_t = epool.tile([128, TB, COUT], f32, tag="o")
                    nc.vector.tensor_reduce(
                        out=o_t[:, :nb, :],
                        in_=msel_v.rearrange("p b k d -> p b d k"),
                        axis=mybir.AxisListType.X,
                        op=mybir.AluOpType.add,
                    )
                    nc.vector.tensor_scalar_max(o_t[:, :nb, :], o_t[:, :nb, :], 0.0)
                    orows = slice(b0 * TPT, (it + 1) * TPT)
                    nc.sync.dma_start(
                        out=out_d.ap()[orows, :].rearrange(
                            "(t p) c -> p t c", p=128),
                        in_=o_t[:, :nb, :])

    nc.compile()
    return nc


def make_idx16(idx_vp, ntiles, base):
    """[vpad, E] int32 row indices -> [ntiles*128, NS] wrapped int16.

    Queue q's gather list (3968 entries) for tile t: position n = s*128 + p
    holds idx[t*128+p, q*30+s] - base for s < 30, and +1 (pad row base+1) for
    s == 30 so the ucode's trailing-negative trim never drops real rows.
    Each queue's list is wrapped [16, 248] and replicated across the 8
    16-partition groups.
    """
    nt = ntiles
    arr = np.full((nt, NQ, SQP, 128), 1, np.int32)
    blk = idx_vp[:nt * TPT].reshape(nt, TPT, NQ, SQ)  # [t, p, q, s]
    arr[:, :, :SQ, :] = blk.transpose(0, 2, 3, 1) - base
    # wrap each queue's flat list (s-major) into [16, 248]
    w = arr.reshape(nt, NQ, SQP * 128 // 16, 16).transpose(0, 1, 3, 2)  # [t,q,16,248]
    w2 = w.transpose(0, 2, 1, 3).reshape(nt, 16, NQ * NSQ)  # [t, 16, NS]
    out = np.tile(w2, (1, 8, 1)).astype(np.int16)  # [t, 128, NS]
    return out.reshape(nt * 128, NS)


def _host_prep(signal, bary_w, bary_idx, kernel):
    """Build per-core input maps. All host-side numpy, not timed."""
    jj = np.arange(NT)
    rot = kernel[:, (jj[:, None] + jj[None, :]) % NT, :, :]  # [i,j,k,c,d]
    wm = np.ascontiguousarray(
        rot.transpose(0, 1, 3, 2, 4).reshape(KC, ND), dtype=np.float32
    )
    sig = np.ascontiguousarray(signal, dtype=np.float32)
    base = 32768
    wv_full = bary_w.reshape(V, E).astype(np.float32)
    idx_full = bary_idx.reshape(V, E).astype(np.int32)
    in_maps = []
    for c in range(NCORES):
        sl = slice(c * VPC, (c + 1) * VPC)
        wv = np.zeros((VPAD, E), np.float32)
        wv[:VPC] = wv_full[sl]
        idx = np.zeros((VPAD, E), np.int32)
        idx[:VPC] = idx_full[sl]
        in_maps.append({
            "signal": sig,
            "wv": wv,
            "idx16": make_idx16(idx, NTILES, base),
            "wm": wm,
        })
    return in_maps


def kernel(signal, bary_w, bary_idx, kernel):
    from concourse.bass_utils import run_bass_kernel_spmd

    if "nc" not in _CACHE:
        _CACHE["nc"] = build_program()
    nc = _CACHE["nc"]
    in_maps = _host_prep(signal, bary_w, bary_idx, kernel)
    res = run_bass_kernel_spmd(nc, in_maps, core_ids=list(range(NCORES)))
    out = np.concatenate(
        [res.results[c]["out"][:VPC] for c in range(NCORES)], axis=0
    )
    return out.astype(np.float32)
